# revision 43
# baseline (speedup 1.0000x reference)
"""CTC loss (reduction='mean', zero_infinity) on 8 Trainium2 NeuronCores.

Strategy (data-parallel over batch, 8 batch elems per core):
  - Stream logits tiles (128 rows = 8b x 16t, 1296 cols) HBM->SBUF once,
    split across two HWDGE load queues (SP / ACT).
  - ACT: e = exp(x - 1) in bf16 with free-dim accumulation -> per-(b,t)
    sum S' = S/e into a [128, 32] buffer; a single Ln pass at the end
    (avoids per-tile Exp<->Ln activation-table reload thrash).
  - GPSIMD ap_gather picks the 65 extended-label columns per row (forward
    index list for tiles 0-15, s-reversed list for tiles 16-31).  Gathers
    for a group of tiles stack into one SBUF buffer; ONE DMA per group
    moves them into the p-hat store (grouped writes amortize the ~3us
    fixed cost per DMA; graduated group sizes so the DP starts early).
  - P-hat store PH4 [16, dt, tile*80+s] bf16: partitions 0-7 hold
    ph[b, t, s] for t < 256 (forward chain), partitions 8-15 hold
    ph[b, 511-u, 64-s] at position u (backward chain, time- and lattice-
    reversed so both DP chains read ONE AP per step).  Tiles 0-15 feed
    only the forward half, tiles 16-31 only the backward half.
  - Paired CTC DP in bf16 (DVE 2x mode): forward alpha (partitions 0-7)
    and backward delta (partitions 8-15, s-reversed so its stencil matches
    forward's) advance in lockstep: 255 steps x 4 DVE tensor ops on
    [16, 65] tiles, renorm every 32 steps anchoring the max at e^+60
    (renorm maxima batched into one Ln at the end).
  - Junction at t*=255: gamma* from delta (3 ops); alpha, gamma*, renorm
    logs and ln-sum-S ship to the host, which does the final 65-wide
    log-sum-exp in float64 (the ACT Ln LUT saturates near 1.2e-20, which
    would corrupt a device-side LSE) and the mean(nll/target_len).
"""
import numpy as np

import concourse.bass as bass
import concourse.bacc as bacc
import concourse.mybir as mybir
import concourse.tile as tile
from concourse.bass_utils import run_bass_kernel_spmd

f32 = mybir.dt.float32
bf16 = mybir.dt.bfloat16
u16 = mybir.dt.uint16
AF = mybir.ActivationFunctionType
ALU = mybir.AluOpType
AX = mybir.AxisListType

B, T, V, S = 64, 512, 1296, 32
L = 2 * S + 1          # 65
NCORES = 8
BL = B // NCORES       # 8 batch elems per core
W = 80                 # gather cols per row: 65 used + pad (16-mult)
CHUNK = 16             # time steps per memory tile (128 rows / 8 b)
NK = T // CHUNK        # 32 tiles
TSTAR = (T - 2) // 2   # 255: paired chains, fwd t=1..255, bwd t=510..256
RENORM = 32
NRE = TSTAR // RENORM  # 7 renorm events
KLN = 60.0             # renorm anchor: max -> e^KLN

# tile groups: (is_fwd, [tile indices in gather order]); small leading groups
# so the DP can start early, larger ones after.  fwd tiles ascend, bwd tiles
# descend (natural consumption order of each chain).
GROUPS = [
    (True, [0]), (False, [31]), (True, [1]), (False, [30]),
    (True, [2, 3]), (False, [29, 28]), (True, [4, 5]), (False, [27, 26]),
    (True, [6, 7]), (False, [25, 24]),
    (True, [8, 9, 10, 11]), (False, [23, 22, 21, 20]),
    (True, [12, 13, 14, 15]), (False, [19, 18, 17, 16]),
]
GMAX = max(len(g[1]) for g in GROUPS)


def _body(nc, tc, lg, idx, msk, outt):
    NR = 2 * NRE                   # 14 per-sample renorm scale factors
    KF = float(np.exp(np.float32(KLN)))
    SC = float(2.0 ** -64)         # keep Ln inputs inside the ACT domain

    with tc.tile_pool(name="const", bufs=1) as cpool, \
         tc.tile_pool(name="lt", bufs=6) as lpool, \
         tc.tile_pool(name="et", bufs=4) as epool, \
         tc.tile_pool(name="gt", bufs=3) as gpool, \
         tc.tile_pool(name="dp", bufs=3) as dpool:

        # [2*8b, dt, tile*W+c] (cols 0:65 of each W-block used); fwd half:
        # t = tile*16+dt; bwd half: u = same, real t = 511-u, s reversed
        # (the reversal is baked into the bwd gather index list).  Two
        # separate tiles (DP steps 1-127 / 128-255) so dependency tracking
        # never gates first-half DP steps on second-half writes.
        PH0 = cpool.tile([2 * BL, CHUNK, (NK // 8) * W], bf16, tag="PH0")
        PH1 = cpool.tile([2 * BL, CHUNK, (NK // 8) * W], bf16, tag="PH1")
        PH2 = cpool.tile([2 * BL, CHUNK, (NK // 8) * W], bf16, tag="PH2")
        PH3 = cpool.tile([2 * BL, CHUNK, (NK // 8) * W], bf16, tag="PH3")
        PHS = [PH0, PH1, PH2, PH3]
        idxF = cpool.tile([128, W // 16], u16, tag="idxF")
        nc.sync.dma_start(idxF[:], idx[:, 0:W // 16])
        idxB = cpool.tile([128, W // 16], u16, tag="idxB")
        nc.sync.dma_start(idxB[:], idx[:, W // 16:2 * (W // 16)])
        M_sb = cpool.tile([2 * BL, L], f32, tag="M")
        nc.sync.dma_start(M_sb[:], msk)
        M16 = cpool.tile([2 * BL, L], bf16, tag="M16")
        nc.vector.tensor_scalar_mul(M16[:], M_sb[:], 1.0)
        bm1 = cpool.tile([128, 1], f32, tag="bm1")
        nc.vector.memset(bm1[:], -1.0)
        SAcc = cpool.tile([128, NK], f32, tag="SAcc")

        # ---- memory phase: grouped gathers + one PH write per group
        # (fwd groups on the SP HWDGE queue, bwd on the GPSIMD SWDGE queue)
        qtoggle = 0
        for is_fwd, tiles in GROUPS:
            G = len(tiles)
            gts = gpool.tile([128, GMAX * W], bf16, tag="gts")
            idx_sb = idxF if is_fwd else idxB
            for g, k in enumerate(tiles):
                lt = lpool.tile([128, V], f32, tag="lt")
                srcap = lg[:, k * CHUNK:(k + 1) * CHUNK, :]
                ldq = nc.scalar if qtoggle % 3 == 1 else nc.sync
                qtoggle += 1
                ldq.dma_start(lt[:], srcap)
                et = epool.tile([128, V], bf16, tag="et")
                nc.scalar.activation(et[:], lt[:], AF.Exp, bias=bm1[:],
                                     accum_out=SAcc[:, k:k + 1])
                nc.gpsimd.indirect_copy(gts[:, g * W:(g + 1) * W], et[:],
                                        idx_sb[:], True)
            if is_fwd:
                a0 = tiles[0]  # ascending run: a-index == tile index
                PH = PHS[a0 // 4]
                a0 %= 4
                dst = PH[0:BL, :, a0 * W:(a0 + G) * W]
                nc.sync.dma_start(dst, gts[:, 0:G * W])
            else:
                a0 = 31 - tiles[0]  # descending run: a-index = 31 - tile
                PH = PHS[a0 // 4]
                a0 %= 4
                dst = PH[BL:2 * BL, CHUNK - 1::-1, a0 * W:(a0 + G) * W]
                nc.gpsimd.dma_start(dst, gts[:, 0:G * W])

        # ---- paired DP on DVE: state cols 2:L+2, zero guards at 0:2
        alA = cpool.tile([2 * BL, L + 2], bf16, tag="alA")
        alB = cpool.tile([2 * BL, L + 2], bf16, tag="alB")
        nc.vector.memset(alA[:], 0.0)
        nc.vector.memset(alB[:], 0.0)
        MXS = cpool.tile([2 * BL, NRE], f32, tag="MXS")
        nc.vector.tensor_scalar_mul(alA[:, 2:4], PHS[0][:, 0, 0:2], KF)
        cur, nxt = alA, alB
        for t in range(1, TSTAR + 1):
            a = t // CHUNK
            PH = PHS[a // 4]
            a %= 4
            ph = PH[:, t % CHUNK, a * W:a * W + L]
            u = dpool.tile([2 * BL, L], bf16, tag="u")
            nc.vector.tensor_tensor(u[:], cur[:, 2:L + 2], cur[:, 1:L + 1], op=ALU.add)
            v = dpool.tile([2 * BL, L], bf16, tag="v")
            nc.vector.tensor_tensor(v[:], cur[:, 0:L], M16[:], op=ALU.mult)
            nc.vector.tensor_tensor(u[:], u[:], v[:], op=ALU.add)
            nc.vector.tensor_tensor(nxt[:, 2:L + 2], u[:], ph, op=ALU.mult)
            cur, nxt = nxt, cur
            if t % RENORM == 0:
                i = t // RENORM - 1
                mx = MXS[:, i:i + 1]
                nc.vector.tensor_reduce(mx, cur[:, 2:L + 2], axis=AX.X, op=ALU.max)
                nc.vector.tensor_scalar_max(mx, mx, 1e-30)
                rc = dpool.tile([2 * BL, 1], f32, tag="rc")
                nc.vector.reciprocal(rc[:], mx)
                nc.vector.tensor_scalar(cur[:, 2:L + 2], cur[:, 2:L + 2], rc[:], KF,
                                        op0=ALU.mult, op1=ALU.mult)
        # fwd half of cur = alpha_255; bwd half = delta_256 (s-reversed)

        # gamma* = de + sh1(de) + sh2(de)*M2~ (no p-hat multiply); fwd half junk
        gam = cpool.tile([2 * BL, L], bf16, tag="gam")
        nc.vector.tensor_tensor(gam[:], cur[:, 2:L + 2], cur[:, 1:L + 1], op=ALU.add)
        gv = dpool.tile([2 * BL, L], bf16, tag="gv")
        nc.vector.tensor_tensor(gv[:], cur[:, 0:L], M16[:], op=ALU.mult)
        nc.vector.tensor_tensor(gam[:], gam[:], gv[:], op=ALU.add)

        # renorm scale logs, batched: Cacc[p] = sum_i ln(mx_i * SC)
        lnmx = cpool.tile([2 * BL, NRE], f32, tag="lnmx")
        nc.scalar.activation(lnmx[:], MXS[:], AF.Ln, scale=SC)
        Cacc = cpool.tile([2 * BL, 1], f32, tag="Cacc")
        nc.vector.tensor_reduce(Cacc[:], lnmx[:], axis=AX.X, op=ALU.add)

        # per-b sum of ln S': Ln over [128, NK], reduce, regroup (8,16), reduce
        lns32 = cpool.tile([128, NK], f32, tag="lns32")
        nc.scalar.activation(lns32[:], SAcc[:], AF.Ln)
        red = cpool.tile([128, 1], f32, tag="red")
        nc.vector.tensor_reduce(red[:], lns32[:], axis=AX.X, op=ALU.add)
        sb16 = cpool.tile([BL, 16], f32, tag="sb16")
        nc.sync.dma_start(sb16[:], red[:])
        sb1 = cpool.tile([BL, 1], f32, tag="sb1")
        nc.vector.tensor_reduce(sb1[:], sb16[:], axis=AX.X, op=ALU.add)

        # ---- output: the final 65-wide LSE junction runs on the host in
        # float64 (the ACT Ln LUT saturates near 1.2e-20, corrupting a
        # device-side LSE).  Ship alpha, gamma (still s-reversed), Cacc, sb1.
        ob = cpool.tile([2 * BL, L + 2], f32, tag="ob")
        nc.vector.tensor_scalar_mul(ob[:, 0:1], Cacc[:], 1.0)
        nc.vector.tensor_scalar_mul(ob[0:BL, 1:2], sb1[:], 1.0)
        nc.vector.tensor_scalar_mul(ob[:, 2:L + 2], gam[:], 1.0)
        nc.vector.tensor_scalar_mul(ob[0:BL, 2:L + 2], cur[0:BL, 2:L + 2], 1.0)
        nc.sync.dma_start(outt, ob[:])


KERNEL_VER = 19


def build_bass():
    nc = bacc.Bacc("TRN2")
    # dummy input whose shape encodes the kernel version: busts stale
    # HLO-hash-keyed executable caches when the BIR changes
    ver = nc.dram_tensor("ver", (1, KERNEL_VER), f32, kind="ExternalInput")
    lg = nc.dram_tensor("logits", (BL, T, V), f32, kind="ExternalInput")
    idx = nc.dram_tensor("idx", (128, 2 * (W // 16)), u16, kind="ExternalInput")
    msk = nc.dram_tensor("mask", (2 * BL, L), f32, kind="ExternalInput")
    outt = nc.dram_tensor("out", (2 * BL, L + 2), f32, kind="ExternalOutput")
    with tile.TileContext(nc) as tc:
        with tc.tile_pool(name="ver", bufs=1) as vpool:
            vt = vpool.tile([1, KERNEL_VER], f32)
            nc.sync.dma_start(vt[:], ver.ap())
        _body(nc, tc, lg.ap(), idx.ap(), msk.ap(), outt.ap())
    nc.compile()
    return nc


def host_prep(targets):
    """Per-core gather indices (wrapped) and the paired skip mask."""
    targets = np.asarray(targets).astype(np.int64)
    ext = np.zeros((B, L), dtype=np.int64)
    ext[:, 1::2] = targets
    pos = np.arange(L)
    ext_m2 = np.full((B, L), -1, dtype=np.int64)
    ext_m2[:, 2:] = ext[:, :-2]
    M = ((pos[None, :] % 2 == 1) & (ext != ext_m2)).astype(np.float32)
    M2 = np.zeros_like(M)
    M2[:, :-2] = M[:, 2:]
    idxs, msks = [], []
    for c in range(NCORES):
        sl = slice(c * BL, (c + 1) * BL)
        def wrap(lst):
            full = np.zeros((BL, W), dtype=np.uint16)
            full[:, :L] = lst.astype(np.uint16)
            return full.reshape(BL, W // 16, 16).transpose(0, 2, 1).reshape(128, W // 16)
        idx_w = np.concatenate([wrap(ext[sl]), wrap(ext[sl, ::-1])], axis=1)
        idxs.append(np.ascontiguousarray(idx_w))
        # paired mask: fwd M on partitions 0-7, s-reversed M2 on 8-15
        mp = np.concatenate([M[sl], M2[sl, ::-1]], axis=0).astype(np.float32)
        msks.append(np.ascontiguousarray(mp))
    return idxs, msks


_nc_cache = {}


def kernel(logits, targets, input_lengths, target_lengths):
    logits = np.ascontiguousarray(np.asarray(logits), dtype=np.float32)
    targets = np.asarray(targets)
    il = np.asarray(input_lengths)
    tl = np.asarray(target_lengths)
    assert logits.shape == (B, T, V)
    assert int(il.min()) == T and int(il.max()) == T, "kernel specialized to full input_lengths"
    assert int(tl.min()) == S and int(tl.max()) == S, "kernel specialized to full target_lengths"

    if "nc" not in _nc_cache:
        _nc_cache["nc"] = build_bass()
    nc = _nc_cache["nc"]

    idxs, msks = host_prep(targets)
    in_maps = []
    for c in range(NCORES):
        sl = slice(c * BL, (c + 1) * BL)
        in_maps.append({
            "ver": np.zeros((1, KERNEL_VER), dtype=np.float32),
            "logits": np.ascontiguousarray(logits[sl]),
            "idx": idxs[c],
            "mask": msks[c],
        })
    res = run_bass_kernel_spmd(nc, in_maps, core_ids=list(range(NCORES)))
    # host epilogue (float64): per-sample 65-wide LSE junction + constants
    NR = 2 * NRE
    cst = 16.0 * KLN - (64.0 * NR) * np.log(2.0)
    nlls = []
    for c in range(NCORES):
        o = np.asarray(res.results[c]["out"]).astype(np.float64)  # (16, 67)
        CaccF, CaccB = o[0:BL, 0], o[BL:2 * BL, 0]
        sb1 = o[0:BL, 1]
        alpha = o[0:BL, 2:L + 2]
        gamma = o[BL:2 * BL, 2:L + 2][:, ::-1]  # un-reverse s
        with np.errstate(divide="ignore"):
            qq = np.log(alpha) + np.log(gamma)
        mq = np.max(qq, axis=1)
        safe = np.isfinite(mq)
        lse = np.where(
            safe,
            mq + np.log(np.exp(qq - np.where(safe, mq, 0.0)[:, None]).sum(axis=1)),
            -np.inf)
        nlls.append(-lse - CaccF - CaccB + sb1 + cst)
    nll = np.concatenate(nlls)
    ok = np.isfinite(nll) & (nll < 1e29)
    nll = np.where(ok, nll, 0.0)
    return np.float32(np.mean(nll / tl.astype(np.float64)))


# revision 45
# speedup vs baseline: 1.0686x; 1.0686x over previous
"""CTC loss (reduction='mean', zero_infinity) on 8 Trainium2 NeuronCores.

Strategy (data-parallel over batch, 8 batch elems per core):
  - Stream logits tiles (128 rows = 8b x 16t, 1296 cols) HBM->SBUF once,
    split across two HWDGE load queues (SP / ACT).
  - ACT: e = exp(x - 1) in bf16 with free-dim accumulation -> per-(b,t)
    sum S' = S/e into a [128, 32] buffer; a single Ln pass at the end
    (avoids per-tile Exp<->Ln activation-table reload thrash).
  - GPSIMD ap_gather picks the 65 extended-label columns per row (forward
    index list for tiles 0-15, s-reversed list for tiles 16-31).  Gathers
    for a group of tiles stack into one SBUF buffer; ONE DMA per group
    moves them into the p-hat store (grouped writes amortize the ~3us
    fixed cost per DMA; graduated group sizes so the DP starts early).
  - P-hat store PH4 [16, dt, tile*80+s] bf16: partitions 0-7 hold
    ph[b, t, s] for t < 256 (forward chain), partitions 8-15 hold
    ph[b, 511-u, 64-s] at position u (backward chain, time- and lattice-
    reversed so both DP chains read ONE AP per step).  Tiles 0-15 feed
    only the forward half, tiles 16-31 only the backward half.
  - Paired CTC DP in bf16 (DVE 2x mode): forward alpha (partitions 0-7)
    and backward delta (partitions 8-15, s-reversed so its stencil matches
    forward's) advance in lockstep: 255 steps x 4 DVE tensor ops on
    [16, 65] tiles, renorm every 32 steps anchoring the max at e^+60
    (renorm maxima batched into one Ln at the end).
  - Junction at t*=255: gamma* from delta (3 ops); alpha, gamma*, renorm
    logs and ln-sum-S ship to the host, which does the final 65-wide
    log-sum-exp in float64 (the ACT Ln LUT saturates near 1.2e-20, which
    would corrupt a device-side LSE) and the mean(nll/target_len).
"""
import numpy as np

import concourse.bass as bass
import concourse.bacc as bacc
import concourse.mybir as mybir
import concourse.tile as tile
from concourse.bass_utils import run_bass_kernel_spmd

f32 = mybir.dt.float32
bf16 = mybir.dt.bfloat16
u16 = mybir.dt.uint16
AF = mybir.ActivationFunctionType
ALU = mybir.AluOpType
AX = mybir.AxisListType

B, T, V, S = 64, 512, 1296, 32
L = 2 * S + 1          # 65
NCORES = 8
BL = B // NCORES       # 8 batch elems per core
W = 80                 # gather cols per row: 65 used + pad (16-mult)
CHUNK = 16             # time steps per memory tile (128 rows / 8 b)
NK = T // CHUNK        # 32 tiles
TSTAR = (T - 2) // 2   # 255: paired chains, fwd t=1..255, bwd t=510..256
RENORM = 32
NRE = TSTAR // RENORM  # 7 renorm events
KLN = 60.0             # renorm anchor: max -> e^KLN

# tile groups: (is_fwd, [tile indices in gather order]); small leading groups
# so the DP can start early, larger ones after.  fwd tiles ascend, bwd tiles
# descend (natural consumption order of each chain).
GROUPS = [
    (True, [0]), (False, [31]), (True, [1]), (False, [30]),
    (True, [2, 3]), (False, [29, 28]), (True, [4, 5]), (False, [27, 26]),
    (True, [6, 7]), (False, [25, 24]),
    (True, [8, 9, 10, 11]), (False, [23, 22, 21, 20]),
    (True, [12, 13, 14, 15]), (False, [19, 18, 17, 16]),
]
GMAX = max(len(g[1]) for g in GROUPS)


def _body(nc, tc, lg, idx, msk, outt):
    NR = 2 * NRE                   # 14 per-sample renorm scale factors
    KF = float(np.exp(np.float32(KLN)))
    SC = float(2.0 ** -64)         # keep Ln inputs inside the ACT domain

    with tc.tile_pool(name="const", bufs=1) as cpool, \
         tc.tile_pool(name="lt", bufs=6) as lpool, \
         tc.tile_pool(name="et", bufs=4) as epool, \
         tc.tile_pool(name="gt", bufs=3) as gpool, \
         tc.tile_pool(name="dp", bufs=3) as dpool:

        # [2*8b, dt, tile*W+c] (cols 0:65 of each W-block used); fwd half:
        # t = tile*16+dt; bwd half: u = same, real t = 511-u, s reversed
        # (the reversal is baked into the bwd gather index list).  Two
        # separate tiles (DP steps 1-127 / 128-255) so dependency tracking
        # never gates first-half DP steps on second-half writes.
        PH0 = cpool.tile([2 * BL, CHUNK, (NK // 8) * W], bf16, tag="PH0")
        PH1 = cpool.tile([2 * BL, CHUNK, (NK // 8) * W], bf16, tag="PH1")
        PH2 = cpool.tile([2 * BL, CHUNK, (NK // 8) * W], bf16, tag="PH2")
        PH3 = cpool.tile([2 * BL, CHUNK, (NK // 8) * W], bf16, tag="PH3")
        PHS = [PH0, PH1, PH2, PH3]
        idxF = cpool.tile([128, W // 16], u16, tag="idxF")
        nc.sync.dma_start(idxF[:], idx[:, 0:W // 16])
        idxB = cpool.tile([128, W // 16], u16, tag="idxB")
        nc.sync.dma_start(idxB[:], idx[:, W // 16:2 * (W // 16)])
        M_sb = cpool.tile([2 * BL, L], f32, tag="M")
        nc.sync.dma_start(M_sb[:], msk)
        M16 = cpool.tile([2 * BL, L], bf16, tag="M16")
        nc.vector.tensor_scalar_mul(M16[:], M_sb[:], 1.0)
        bm1 = cpool.tile([128, 1], f32, tag="bm1")
        nc.vector.memset(bm1[:], -1.0)
        SAcc = cpool.tile([128, NK], f32, tag="SAcc")

        # ---- memory phase: grouped gathers + one PH write per group
        # (fwd groups on the SP HWDGE queue, bwd on the GPSIMD SWDGE queue)
        qtoggle = 0
        for is_fwd, tiles in GROUPS:
            G = len(tiles)
            gts = gpool.tile([128, GMAX * W], bf16, tag="gts")
            idx_sb = idxF if is_fwd else idxB
            for g, k in enumerate(tiles):
                lt = lpool.tile([128, V], f32, tag="lt")
                srcap = lg[:, k * CHUNK:(k + 1) * CHUNK, :]
                ldq = nc.scalar if qtoggle % 3 == 1 else nc.sync
                qtoggle += 1
                ldq.dma_start(lt[:], srcap)
                et = epool.tile([128, V], bf16, tag="et")
                nc.scalar.activation(et[:], lt[:], AF.Exp, bias=bm1[:],
                                     accum_out=SAcc[:, k:k + 1])
                nc.gpsimd.indirect_copy(gts[:, g * W:(g + 1) * W], et[:],
                                        idx_sb[:], True)
            if is_fwd:
                a0 = tiles[0]  # ascending run: a-index == tile index
                PH = PHS[a0 // 4]
                a0 %= 4
                dst = PH[0:BL, :, a0 * W:(a0 + G) * W]
                nc.sync.dma_start(dst, gts[:, 0:G * W])
            else:
                a0 = 31 - tiles[0]  # descending run: a-index = 31 - tile
                PH = PHS[a0 // 4]
                a0 %= 4
                dst = PH[BL:2 * BL, CHUNK - 1::-1, a0 * W:(a0 + G) * W]
                nc.gpsimd.dma_start(dst, gts[:, 0:G * W])

        # ---- paired DP on DVE: state cols 2:L+2, zero guards at 0:2
        alA = cpool.tile([2 * BL, L + 2], bf16, tag="alA")
        alB = cpool.tile([2 * BL, L + 2], bf16, tag="alB")
        nc.vector.memset(alA[:], 0.0)
        nc.vector.memset(alB[:], 0.0)
        MXS = cpool.tile([2 * BL, NRE], f32, tag="MXS")
        nc.vector.tensor_scalar_mul(alA[:, 2:4], PHS[0][:, 0, 0:2], KF)
        cur, nxt = alA, alB
        for t in range(1, TSTAR + 1):
            a = t // CHUNK
            PH = PHS[a // 4]
            a %= 4
            ph = PH[:, t % CHUNK, a * W:a * W + L]
            u = dpool.tile([2 * BL, L], bf16, tag="u")
            nc.vector.tensor_tensor(u[:], cur[:, 2:L + 2], cur[:, 1:L + 1], op=ALU.add)
            v = dpool.tile([2 * BL, L], bf16, tag="v")
            nc.vector.tensor_tensor(v[:], cur[:, 0:L], M16[:], op=ALU.mult)
            nc.vector.tensor_tensor(u[:], u[:], v[:], op=ALU.add)
            nc.vector.tensor_tensor(nxt[:, 2:L + 2], u[:], ph, op=ALU.mult)
            cur, nxt = nxt, cur
            if t % RENORM == 0:
                i = t // RENORM - 1
                mx = MXS[:, i:i + 1]
                nc.vector.tensor_reduce(mx, cur[:, 2:L + 2], axis=AX.X, op=ALU.max)
                nc.vector.tensor_scalar_max(mx, mx, 1e-30)
                rc = dpool.tile([2 * BL, 1], f32, tag="rc")
                nc.vector.reciprocal(rc[:], mx)
                nc.vector.tensor_scalar(cur[:, 2:L + 2], cur[:, 2:L + 2], rc[:], KF,
                                        op0=ALU.mult, op1=ALU.mult)
        # fwd half of cur = alpha_255; bwd half = delta_256 (s-reversed)

        # gamma* = de + sh1(de) + sh2(de)*M2~ (no p-hat multiply); fwd half junk
        gam = cpool.tile([2 * BL, L], bf16, tag="gam")
        nc.vector.tensor_tensor(gam[:], cur[:, 2:L + 2], cur[:, 1:L + 1], op=ALU.add)
        gv = dpool.tile([2 * BL, L], bf16, tag="gv")
        nc.vector.tensor_tensor(gv[:], cur[:, 0:L], M16[:], op=ALU.mult)
        nc.vector.tensor_tensor(gam[:], gam[:], gv[:], op=ALU.add)

        # renorm scale logs, batched: Cacc[p] = sum_i ln(mx_i * SC)
        lnmx = cpool.tile([2 * BL, NRE], f32, tag="lnmx")
        nc.scalar.activation(lnmx[:], MXS[:], AF.Ln, scale=SC)
        Cacc = cpool.tile([2 * BL, 1], f32, tag="Cacc")
        nc.vector.tensor_reduce(Cacc[:], lnmx[:], axis=AX.X, op=ALU.add)

        # per-b sum of ln S': Ln over [128, NK], reduce, regroup (8,16), reduce
        lns32 = cpool.tile([128, NK], f32, tag="lns32")
        nc.scalar.activation(lns32[:], SAcc[:], AF.Ln)
        red = cpool.tile([128, 1], f32, tag="red")
        nc.vector.tensor_reduce(red[:], lns32[:], axis=AX.X, op=ALU.add)
        sb16 = cpool.tile([BL, 16], f32, tag="sb16")
        nc.sync.dma_start(sb16[:], red[:])
        sb1 = cpool.tile([BL, 1], f32, tag="sb1")
        nc.vector.tensor_reduce(sb1[:], sb16[:], axis=AX.X, op=ALU.add)

        # ---- output: the final 65-wide LSE junction runs on the host in
        # float64 (the ACT Ln LUT saturates near 1.2e-20, corrupting a
        # device-side LSE).  Ship alpha, gamma (still s-reversed), Cacc, sb1.
        ob = cpool.tile([2 * BL, L + 2], f32, tag="ob")
        nc.vector.tensor_scalar_mul(ob[:, 0:1], Cacc[:], 1.0)
        nc.vector.tensor_scalar_mul(ob[0:BL, 1:2], sb1[:], 1.0)
        nc.vector.tensor_scalar_mul(ob[:, 2:L + 2], gam[:], 1.0)
        nc.vector.tensor_scalar_mul(ob[0:BL, 2:L + 2], cur[0:BL, 2:L + 2], 1.0)
        nc.sync.dma_start(outt, ob[:])


KERNEL_VER = 20


def build_bass():
    nc = bacc.Bacc("TRN2")
    # dummy input whose shape encodes the kernel version: busts stale
    # HLO-hash-keyed executable caches when the BIR changes
    ver = nc.dram_tensor("ver", (1, KERNEL_VER), f32, kind="ExternalInput")
    lg = nc.dram_tensor("logits", (BL, T, V), f32, kind="ExternalInput")
    idx = nc.dram_tensor("idx", (128, 2 * (W // 16)), u16, kind="ExternalInput")
    msk = nc.dram_tensor("mask", (2 * BL, L), f32, kind="ExternalInput")
    outt = nc.dram_tensor("out", (2 * BL, L + 2), f32, kind="ExternalOutput")
    with tile.TileContext(nc) as tc:
        with tc.tile_pool(name="ver", bufs=1) as vpool:
            vt = vpool.tile([1, KERNEL_VER], f32)
            nc.sync.dma_start(vt[:], ver.ap())
        _body(nc, tc, lg.ap(), idx.ap(), msk.ap(), outt.ap())
    nc.compile()
    return nc


def host_prep(targets):
    """Per-core gather indices (wrapped) and the paired skip mask."""
    targets = np.asarray(targets).astype(np.int64)
    ext = np.zeros((B, L), dtype=np.int64)
    ext[:, 1::2] = targets
    pos = np.arange(L)
    ext_m2 = np.full((B, L), -1, dtype=np.int64)
    ext_m2[:, 2:] = ext[:, :-2]
    M = ((pos[None, :] % 2 == 1) & (ext != ext_m2)).astype(np.float32)
    M2 = np.zeros_like(M)
    M2[:, :-2] = M[:, 2:]
    idxs, msks = [], []
    for c in range(NCORES):
        sl = slice(c * BL, (c + 1) * BL)
        def wrap(lst):
            full = np.zeros((BL, W), dtype=np.uint16)
            full[:, :L] = lst.astype(np.uint16)
            return full.reshape(BL, W // 16, 16).transpose(0, 2, 1).reshape(128, W // 16)
        idx_w = np.concatenate([wrap(ext[sl]), wrap(ext[sl, ::-1])], axis=1)
        idxs.append(np.ascontiguousarray(idx_w))
        # paired mask: fwd M on partitions 0-7, s-reversed M2 on 8-15
        mp = np.concatenate([M[sl], M2[sl, ::-1]], axis=0).astype(np.float32)
        msks.append(np.ascontiguousarray(mp))
    return idxs, msks


_nc_cache = {}


def kernel(logits, targets, input_lengths, target_lengths):
    logits = np.ascontiguousarray(np.asarray(logits), dtype=np.float32)
    targets = np.asarray(targets)
    il = np.asarray(input_lengths)
    tl = np.asarray(target_lengths)
    assert logits.shape == (B, T, V)
    assert int(il.min()) == T and int(il.max()) == T, "kernel specialized to full input_lengths"
    assert int(tl.min()) == S and int(tl.max()) == S, "kernel specialized to full target_lengths"

    if "nc" not in _nc_cache:
        _nc_cache["nc"] = build_bass()
    nc = _nc_cache["nc"]

    idxs, msks = host_prep(targets)
    in_maps = []
    for c in range(NCORES):
        sl = slice(c * BL, (c + 1) * BL)
        in_maps.append({
            "ver": np.zeros((1, KERNEL_VER), dtype=np.float32),
            "logits": np.ascontiguousarray(logits[sl]),
            "idx": idxs[c],
            "mask": msks[c],
        })
    res = run_bass_kernel_spmd(nc, in_maps, core_ids=list(range(NCORES)))
    # host epilogue (float64): per-sample 65-wide LSE junction + constants
    NR = 2 * NRE
    cst = 16.0 * KLN - (64.0 * NR) * np.log(2.0)
    nlls = []
    for c in range(NCORES):
        o = np.asarray(res.results[c]["out"]).astype(np.float64)  # (16, 67)
        CaccF, CaccB = o[0:BL, 0], o[BL:2 * BL, 0]
        sb1 = o[0:BL, 1]
        alpha = o[0:BL, 2:L + 2]
        gamma = o[BL:2 * BL, 2:L + 2][:, ::-1]  # un-reverse s
        with np.errstate(divide="ignore"):
            qq = np.log(alpha) + np.log(gamma)
        mq = np.max(qq, axis=1)
        safe = np.isfinite(mq)
        lse = np.where(
            safe,
            mq + np.log(np.exp(qq - np.where(safe, mq, 0.0)[:, None]).sum(axis=1)),
            -np.inf)
        nlls.append(-lse - CaccF - CaccB + sb1 + cst)
    nll = np.concatenate(nlls)
    ok = np.isfinite(nll) & (nll < 1e29)
    nll = np.where(ok, nll, 0.0)
    return np.float32(np.mean(nll / tl.astype(np.float64)))
